# revision 3
# baseline (speedup 1.0000x reference)
"""GNN decoder kernel for Trainium2 (8 NeuronCores, SPMD data-parallel over graphs).

Computation (see reference):
    offsets[g] = first global node index of graph g (from sorted batch_ids)
    gi[g,e]    = clip(offsets[g] + targets[g,e], 0, N-1)
    q[g]       = concat(emb[gi[g,0]], emb[gi[g,1]])          # [B, 512]
    out        = q @ W + b                                    # [B, 128]

v2 design, per core (512 graphs, 1024 gathered rows):
  - Host casts the per-core embedding slice (32768 rows) to bf16 and
    computes local int16 indices; W is cast to bf16.
  - ONE dma_gather (mlp Q7 library) fetches all 1024 rows transposed:
    qt[p, c, i] = emb_row_i[c*128 + p] -- features land on partitions, so
    no PE transposes are needed at all.  (The baseline paid 8 indirect
    DMAs x ~1.1us Q7 descriptor-gen fixed cost + 16 fp32 PE transposes;
    SWDGE cost model: 994ns fixed + 0.34ns/descriptor, so one 1024-index
    gather costs ~1.3us instead of ~8.8us.)
  - 16 bf16 matmuls (graphs on PSUM partitions, K=512 in 4 chunks per
    128-graph block) accumulate q @ W; bf16 is single-pass on PE vs
    LOW_HIGH double-pass fp32.
  - DVE adds the (f32) bias per graph chunk, one DMA stores the result.
  - All constants + indices ship in ONE packed bf16 tensor (W | bias
    bits | idx bits) loaded with a single HWDGE DMA; bias is bitcast
    back to f32, indices to int16, on the SBUF side.

Raw (non-Tile) engine programs with explicit semaphores, like the
baseline: Tile's entry/exit sync costs ~12us on a kernel this size.

PSUM discipline: po[gc] each get their own bank; DVE reads po[gc] only
after its 4th matmul (s_mm >= gc+1); PE never revisits a bank.
"""

import numpy as np
import ml_dtypes

import concourse.bass as bass
import concourse.bacc as bacc
import concourse.mybir as mybir
from concourse import library_config
from concourse.bass_utils import run_bass_kernel_spmd

N_NODES = 262144
N_GRAPHS = 4096
D = 256            # embedding dim
TS = 128           # target size (output features)
N_CORES = 8
GPC = N_GRAPHS // N_CORES   # 512 graphs per core
RPC = N_NODES // N_CORES    # 32768 embedding rows per core
NIDX = 2 * GPC              # 1024 gathered rows per core
F32 = mybir.dt.float32
BF16 = mybir.dt.bfloat16
I16 = mybir.dt.int16

# packed constants tensor (bf16 [128, 832]) column layout
C_W = 0            # [128, 512]  w[f, fc*128+o] = W[fc*128+f, o]    (bf16)
C_B = 512          # [128, 256]  f32 bias replicated, as bf16 bit-pairs
C_IDX = 768        # [128, 64]   int16 local gather indices, as bf16 bits
C_COLS = 832

# cleared in sim runs: CoreSim's race detector rejects sem_clear-after-drain
# (conservative), while HW needs the teardown for clean NEFF re-execution
TEARDOWN = True


def build_program() -> bass.Bass:
    nc = bacc.Bacc("TRN2", target_bir_lowering=False, debug=False)

    emb = nc.dram_tensor("emb", [RPC, D], BF16, kind="ExternalInput")
    cin = nc.dram_tensor("cin", [128, C_COLS], BF16, kind="ExternalInput")
    out = nc.dram_tensor("out", [GPC, TS], F32, kind="ExternalOutput")

    cin_sb = nc.alloc_sbuf_tensor("cin_sb", [128, C_COLS], BF16)
    qt_sb = nc.alloc_sbuf_tensor("qt_sb", [128, 2, NIDX], BF16)
    out_sb = nc.alloc_sbuf_tensor("o_sb", [128, 4 * TS], F32)

    po = [nc.alloc_psum_tensor(f"po{gc}", [128, TS], F32) for gc in range(4)]

    s_cin = nc.alloc_semaphore("s_cin")
    s_g = nc.alloc_semaphore("s_g")
    s_mm = nc.alloc_semaphore("s_mm")
    s_add = nc.alloc_semaphore("s_add")
    s_out = nc.alloc_semaphore("s_out")

    w_t = cin_sb[:, C_W : C_W + 512]                              # bf16 [128, 512]
    b_t = cin_sb[:, C_B : C_B + 256].bitcast(F32)                 # f32  [128, 128]
    idx_t = cin_sb[:, C_IDX : C_IDX + 64].bitcast(I16)            # i16  [128, 64]

    with nc.Block() as block:

        @block.sync
        def _(sync):
            sync.dma_start(out=cin_sb[:], in_=cin[:, :]).then_inc(s_cin, 16)
            sync.wait_ge(s_add, 4)
            sync.dma_start(
                out=out[:, :].rearrange("(gc p) o -> p gc o", p=128),
                in_=out_sb[:].rearrange("p (gc o) -> p gc o", gc=4),
            ).then_inc(s_out, 16)

        @block.gpsimd
        def _(gpsimd):
            # kick the mlp (dma_gather) Q7 library IRAM load first so it
            # overlaps the cin HWDGE load
            gpsimd.load_library(library_config.mlp)
            gpsimd.wait_ge(s_cin, 16)
            # single_packet=False: coalescing the whole stream into one packet
            # (the default) exceeds the SDMA packet limits at 1024 indices
            # (130 rx descriptors/engine) and kills the NEFF execution on HW
            gpsimd.dma_gather(
                qt_sb[:, :, :],
                emb[:, :],
                idx_t,
                NIDX,
                NIDX,
                D,
                transpose=True,
                single_packet=False,
            ).then_inc(s_g, 16)
            # teardown: zero all semaphores once everything (incl. the output
            # store) completed, so re-executing the loaded NEFF starts clean
            gpsimd.wait_ge(s_out, 16)
            if TEARDOWN:
                gpsimd.dma_reset(range(s_cin.num, s_out.num + 1))
                gpsimd.sem_clear(range(s_cin.num, s_out.num + 1))

        @block.tensor
        def _(tensor):
            tensor.wait_ge(s_g, 16)
            # psum[gc][g, o] += qt[f, c, e*512+gc*128+g] * W[fc*128+f, o]
            for gc in range(4):
                for fc in range(4):
                    e, c = fc >> 1, fc & 1
                    ins = nc.tensor.matmul(
                        out=po[gc][:, 0:TS],
                        lhsT=qt_sb[:, c, e * GPC + gc * 128 : e * GPC + (gc + 1) * 128],
                        rhs=w_t[:, fc * 128 : (fc + 1) * 128],
                        start=(fc == 0),
                        stop=(fc == 3),
                    )
                ins.then_inc(s_mm, 1)

        @block.vector
        def _(vector):
            for gc in range(4):
                vector.wait_ge(s_mm, gc + 1)
                nc.vector.tensor_add(
                    out=out_sb[:, gc * TS : (gc + 1) * TS],
                    in0=po[gc][:, 0:TS],
                    in1=b_t,
                ).then_inc(s_add, 1)

    nc.compile()
    return nc


_PROG = None


def _get_prog() -> bass.Bass:
    global _PROG
    if _PROG is None:
        _PROG = build_program()
    return _PROG


def make_in_maps(batch_emb, batch_ids, targets, W, b):
    emb = np.asarray(batch_emb, dtype=np.float32)
    ids = np.asarray(batch_ids)
    tg = np.asarray(targets)

    # offsets[g] = exclusive prefix count = first index of graph g in sorted ids
    offsets = np.searchsorted(ids, np.arange(N_GRAPHS, dtype=np.int64), side="left")
    gi = offsets[:, None] + tg.astype(np.int64)
    gi = np.clip(gi, 0, N_NODES - 1)  # match jax clamp semantics

    # W reshaped so rhs chunk fc is K-rows fc*128..(fc+1)*128: w[f, fc*128+o]
    w16 = (
        np.asarray(W, dtype=np.float32)
        .reshape(4, 128, TS)
        .transpose(1, 0, 2)
        .reshape(128, 4 * TS)
        .astype(ml_dtypes.bfloat16)
    )
    b_rep = np.broadcast_to(
        np.asarray(b, dtype=np.float32), (128, TS)
    ).astype(np.float32)
    b_bits = np.ascontiguousarray(b_rep).view(np.uint16)  # [128, 256]

    in_maps = []
    for k in range(N_CORES):
        row0 = k * RPC
        emb16 = np.ascontiguousarray(emb[row0 : row0 + RPC]).astype(ml_dtypes.bfloat16)
        # local indices, ordered i = e*GPC + g; int16 (max RPC-1 = 32767)
        blk = gi[k * GPC : (k + 1) * GPC] - row0  # [512, 2]
        blk = np.clip(blk, 0, RPC - 1)
        flat = np.concatenate([blk[:, 0], blk[:, 1]]).astype(np.int16)  # [1024]
        # dma_gather layout: idx[i] at [i % 16, i // 16], replicated to all
        # 8 sixteen-partition groups
        idx16 = flat.reshape(64, 16).T  # [16, 64]
        idx_rep = np.tile(idx16, (8, 1))  # [128, 64]
        cin = np.empty((128, C_COLS), dtype=np.uint16)
        cin[:, C_W : C_W + 512] = w16.view(np.uint16)
        cin[:, C_B : C_B + 256] = b_bits
        cin[:, C_IDX : C_IDX + 64] = idx_rep.view(np.uint16)
        in_maps.append({"emb": emb16, "cin": cin.view(ml_dtypes.bfloat16)})
    return in_maps


def kernel(batch_emb, batch_ids, targets, W, b):
    in_maps = make_in_maps(batch_emb, batch_ids, targets, W, b)
    res = run_bass_kernel_spmd(_get_prog(), in_maps, list(range(N_CORES)))
    return np.concatenate([res.results[k]["out"] for k in range(N_CORES)], axis=0)


# revision 5
# speedup vs baseline: 1.2155x; 1.2155x over previous
"""GNN decoder kernel for Trainium2 (8 NeuronCores, SPMD data-parallel over graphs).

Computation (see reference):
    offsets[g] = first global node index of graph g (from sorted batch_ids)
    gi[g,e]    = clip(offsets[g] + targets[g,e], 0, N-1)
    q[g]       = concat(emb[gi[g,0]], emb[gi[g,1]])          # [B, 512]
    out        = q @ W + b                                    # [B, 128]

v3: baseline's 8-indirect-DMA structure (the resident-ucode SWDGE path is
the fastest way to issue a 128-row gather: ~1.1us fixed per call; the mlp
dma_gather library costs ~6.4us IRAM load + ~8.3ns/idx desc-gen, and the
multi-index indirect form returns garbage on real HW), with the data plane
moved to bf16:
  - host casts the per-core embedding slice and W to bf16 (rel err ~2.4e-3,
    gate is 2e-2); gather traffic halves (64KB per indirect DMA),
  - PE transposes and matmuls run single-pass bf16 instead of LOW_HIGH
    double-pass fp32, so the PE pipeline always keeps up with the gather
    stream instead of adding a ~2.5us tail after the last gather lands,
  - DVE copies cast PSUM f32 -> bf16 for the matmul lhsT,
  - the output store is split into 4 chunk stores issued as each bias-add
    completes, overlapping ~2us of HBM store receipt latency per chunk.

Raw (non-Tile) engine programs with explicit semaphores as in the baseline.
PSUM bank discipline unchanged: transpose banks (ptq) and accumulator banks
(po) are read by DVE only after their 4th PE write; PE never revisits a bank.
"""

import numpy as np
import ml_dtypes

import concourse.bass as bass
import concourse.bacc as bacc
import concourse.mybir as mybir
from concourse.bass_utils import run_bass_kernel_spmd

N_NODES = 262144
N_GRAPHS = 4096
D = 256            # embedding dim
TS = 128           # target size (output features)
N_CORES = 8
GPC = N_GRAPHS // N_CORES   # 512 graphs per core
RPC = N_NODES // N_CORES    # 32768 embedding rows per core
F32 = mybir.dt.float32
BF16 = mybir.dt.bfloat16
I32 = mybir.dt.int32

# constants-tensor column layout (bf16 [128, 896])
C_W = 0            # [128, 512]  w[f, fc*128+o] = W[fc*128+f, o]      (bf16)
C_B = 512          # [128, 256]  f32 bias replicated, as bf16 bit-pairs
C_ID = 768         # [128, 128]  bf16 identity for PE transpose
C_COLS = 896

# cleared in sim runs: CoreSim's race detector rejects sem_clear-after-drain
# (conservative), while HW needs the teardown for clean NEFF re-execution
TEARDOWN = True


def build_program() -> bass.Bass:
    nc = bacc.Bacc("TRN2", target_bir_lowering=False, debug=False)

    emb = nc.dram_tensor("emb", [RPC, D], BF16, kind="ExternalInput")
    idx = nc.dram_tensor("idx", [128, 8], I32, kind="ExternalInput")
    cin = nc.dram_tensor("cin", [128, C_COLS], BF16, kind="ExternalInput")
    out = nc.dram_tensor("out", [GPC, TS], F32, kind="ExternalOutput")

    idx_sb = nc.alloc_sbuf_tensor("idx_sb", [128, 8], I32)
    cin_sb = nc.alloc_sbuf_tensor("cin_sb", [128, C_COLS], BF16)
    g_sb = [nc.alloc_sbuf_tensor(f"g{t}", [128, D], BF16) for t in range(8)]
    qt_sb = [nc.alloc_sbuf_tensor(f"qt{gc}", [128, 512], BF16) for gc in range(4)]
    out_sb = nc.alloc_sbuf_tensor("o_sb", [128, 4 * TS], F32)

    ptq = [nc.alloc_psum_tensor(f"ptq{gc}", [128, 512], BF16) for gc in range(4)]
    po = [nc.alloc_psum_tensor(f"po{gc}", [128, TS], F32) for gc in range(4)]

    s_idx = nc.alloc_semaphore("s_idx")
    s_cin = nc.alloc_semaphore("s_cin")
    s_g = [[nc.alloc_semaphore(f"s_g{e}_{gc}") for gc in range(4)] for e in range(2)]
    s_pe = nc.alloc_semaphore("s_pe")
    s_qt = nc.alloc_semaphore("s_qt")
    s_mm = nc.alloc_semaphore("s_mm")
    s_add = nc.alloc_semaphore("s_add")
    s_out = nc.alloc_semaphore("s_out")

    w_t = cin_sb[:, C_W : C_W + 512]                         # bf16 [128, 512]
    b_t = cin_sb[:, C_B : C_B + 256].bitcast(F32)            # f32  [128, 128]
    ident = cin_sb[:, C_ID : C_ID + 128]                     # bf16 [128, 128]

    with nc.Block() as block:

        @block.sync
        def _(sync):
            sync.dma_start(out=idx_sb[:], in_=idx[:, :]).then_inc(s_idx, 16)
            sync.dma_start(out=cin_sb[:], in_=cin[:, :]).then_inc(s_cin, 16)
            # store each 128-graph chunk as soon as its bias-add lands so the
            # ~2us HBM store receipts overlap compute and each other
            for gc in range(4):
                sync.wait_ge(s_add, gc + 1)
                sync.dma_start(
                    out=out[gc * 128 : (gc + 1) * 128, :],
                    in_=out_sb[:, gc * TS : (gc + 1) * TS],
                ).then_inc(s_out, 16)

        @block.gpsimd
        def _(gpsimd):
            gpsimd.wait_ge(s_idx, 16)
            # pairwise per graph-chunk: (e0,gc), (e1,gc) so PE can finish
            # chunk gc while later chunks still gather
            for gc in range(4):
                for e in range(2):
                    t = e * 4 + gc
                    gpsimd.indirect_dma_start(
                        out=g_sb[t][:],
                        out_offset=None,
                        in_=emb[:, :],
                        in_offset=bass.IndirectOffsetOnAxis(
                            ap=idx_sb[:, t : t + 1], axis=0
                        ),
                    ).then_inc(s_g[e][gc], 16)
            # teardown: zero all semaphores once everything (incl. the output
            # stores) completed, so re-executing the loaded NEFF starts clean
            gpsimd.wait_ge(s_out, 64)
            if TEARDOWN:
                gpsimd.dma_reset(range(s_idx.num, s_out.num + 1))
                gpsimd.sem_clear(range(s_idx.num, s_out.num + 1))

        @block.tensor
        def _(tensor):
            tensor.wait_ge(s_cin, 16)

            def t_half(gc, e, inc=False):
                tensor.wait_ge(s_g[e][gc], 16)
                for c in range(2):
                    fc = 2 * e + c
                    ins = nc.tensor.transpose(
                        out=ptq[gc][:, fc * 128 : (fc + 1) * 128],
                        in_=g_sb[e * 4 + gc][:, c * 128 : (c + 1) * 128],
                        identity=ident,
                    )
                if inc:
                    ins.then_inc(s_pe, 1)

            def t_group(gc):
                t_half(gc, 0)
                t_half(gc, 1, inc=True)

            def m_group(gc):
                tensor.wait_ge(s_qt, gc + 1)
                for fc in range(4):
                    ins = nc.tensor.matmul(
                        out=po[gc][:, 0:TS],
                        lhsT=qt_sb[gc][:, fc * 128 : (fc + 1) * 128],
                        rhs=w_t[:, fc * 128 : (fc + 1) * 128],
                        start=(fc == 0),
                        stop=(fc == 3),
                    )
                ins.then_inc(s_mm, 1)

            t_group(0)
            t_group(1)
            m_group(0)
            t_group(2)
            m_group(1)
            # weave the last chunk so only 2 transposes + 4 matmuls remain
            # after the final gather lands
            t_half(3, 0)
            m_group(2)
            t_half(3, 1, inc=True)
            m_group(3)

        @block.vector
        def _(vector):
            vector.wait_ge(s_cin, 16)

            def c_group(gc):
                vector.wait_ge(s_pe, gc + 1)
                nc.vector.tensor_copy(out=qt_sb[gc][:], in_=ptq[gc][:]).then_inc(
                    s_qt, 1
                )

            def a_group(gc):
                vector.wait_ge(s_mm, gc + 1)
                nc.vector.tensor_add(
                    out=out_sb[:, gc * TS : (gc + 1) * TS],
                    in0=po[gc][:, 0:TS],
                    in1=b_t,
                ).then_inc(s_add, 1)

            c_group(0)
            c_group(1)
            a_group(0)
            c_group(2)
            a_group(1)
            c_group(3)
            a_group(2)
            a_group(3)

    nc.compile()
    return nc


_PROG = None


def _get_prog() -> bass.Bass:
    global _PROG
    if _PROG is None:
        _PROG = build_program()
    return _PROG


def make_in_maps(batch_emb, batch_ids, targets, W, b):
    emb = np.asarray(batch_emb, dtype=np.float32)
    ids = np.asarray(batch_ids)
    tg = np.asarray(targets)

    # offsets[g] = exclusive prefix count = first index of graph g in sorted ids
    offsets = np.searchsorted(ids, np.arange(N_GRAPHS, dtype=np.int64), side="left")
    gi = offsets[:, None] + tg.astype(np.int64)
    gi = np.clip(gi, 0, N_NODES - 1)  # match jax clamp semantics

    w16 = (
        np.asarray(W, dtype=np.float32)
        .reshape(4, 128, TS)
        .transpose(1, 0, 2)
        .reshape(128, 4 * TS)
        .astype(ml_dtypes.bfloat16)
    )
    b_rep = np.ascontiguousarray(
        np.broadcast_to(np.asarray(b, dtype=np.float32), (128, TS))
    )
    ident16 = np.eye(128, dtype=np.float32).astype(ml_dtypes.bfloat16)

    cin = np.empty((128, C_COLS), dtype=np.uint16)
    cin[:, C_W : C_W + 512] = w16.view(np.uint16)
    cin[:, C_B : C_B + 256] = b_rep.view(np.uint16)
    cin[:, C_ID : C_ID + 128] = ident16.view(np.uint16)
    cin16 = cin.view(ml_dtypes.bfloat16)

    in_maps = []
    for k in range(N_CORES):
        row0 = k * RPC
        emb16 = np.ascontiguousarray(emb[row0 : row0 + RPC]).astype(ml_dtypes.bfloat16)
        blk = np.clip(gi[k * GPC : (k + 1) * GPC] - row0, 0, RPC - 1)  # [512, 2]
        idx_k = np.empty((128, 8), np.int32)
        for e in range(2):
            for gc in range(4):
                idx_k[:, e * 4 + gc] = blk[gc * 128 : (gc + 1) * 128, e]
        in_maps.append({"emb": emb16, "idx": idx_k, "cin": cin16})
    return in_maps


def kernel(batch_emb, batch_ids, targets, W, b):
    in_maps = make_in_maps(batch_emb, batch_ids, targets, W, b)
    res = run_bass_kernel_spmd(_get_prog(), in_maps, list(range(N_CORES)))
    return np.concatenate([res.results[k]["out"] for k in range(N_CORES)], axis=0)
